# revision 4
# baseline (speedup 1.0000x reference)
"""Multi-head cross-batch attention (B=4096, d_model=512, H=8 heads) on 8 TRN2 cores.

Sharding: one head per NeuronCore (tensor-parallel over H). Each core computes
its head's Q/K/V projections from a replicated (pre-transposed) x, the full
[4096, 4096] score block for that head, softmax (transposed layout, denominator
via a ones-column in V), attn @ V, and its partial out-projection
Y_h = (attn_h) @ Wo[:, h*64:(h+1)*64].T. Host sums the 8 partials and adds bo.

Layout notes (per core):
  - xT [512, 4096] (c on partitions) is fed from host so every matmul can
    contract over the partition dim without any on-device transpose of x.
  - QT/KT are stored duplicated across partition halves ([128, 4096]) so score
    matmuls can be row-packed two-at-a-time into the 128x128 PE array (the
    contraction dim is only 64).
  - Scores are computed transposed (ST[j, m]) so softmax's sum over keys j can
    ride the attn@V matmul: V is augmented with a ones column, making the
    accumulated output row 64 equal to sum_j exp(s). No max-subtraction is
    needed: scores are O(1) here (verified), so exp cannot overflow.
  - Normalization commutes with the out-projection, so Y_un rows are scaled by
    1/r with a per-partition tensor_scalar after the final matmul. r (living in
    a free-dim row) is transposed to partitions with a K=1 matmul.
"""

import sys

if "/opt/trn_rl_repo" not in sys.path:
    sys.path.insert(0, "/opt/trn_rl_repo")

import numpy as np

import concourse.bass as bass
import concourse.tile as tile
from concourse import bacc, mybir
from concourse.masks import make_identity

B = 4096
D = 512
H = 8
DK = 64
MC = 512  # query-chunk (m) width
N_MC = B // MC  # 8
JB = B // 128  # 32 j-blocks of 128 keys
F32 = mybir.dt.float32

# j-blocks per score/exp group: 3 blocks = 1536 floats = 3 PSUM banks.
# PSUM budget: 2x3 (score staging) + 1 (attnV accum) + 1 (outproj/rT) = 8 banks.
JGROUPS = [(0, 3), (3, 3), (6, 3), (9, 3), (12, 3), (15, 3), (18, 3), (21, 3), (24, 3), (27, 3), (30, 2)]

_NC_CACHE = None


def build_nc():
    nc = bacc.Bacc()

    xt = nc.dram_tensor("xt", [D, B], F32, kind="ExternalInput")
    wqt = nc.dram_tensor("wqt", [D, 128], F32, kind="ExternalInput")  # [c, d dup'd]
    wkt = nc.dram_tensor("wkt", [D, 128], F32, kind="ExternalInput")
    wvt = nc.dram_tensor("wvt", [D, DK], F32, kind="ExternalInput")  # [c, d]
    bqd = nc.dram_tensor("bqd", [128, 1], F32, kind="ExternalInput")  # bias dup'd
    bkd = nc.dram_tensor("bkd", [128, 1], F32, kind="ExternalInput")
    bvr = nc.dram_tensor("bvr", [1, DK], F32, kind="ExternalInput")  # bias as row
    wot = nc.dram_tensor("wot", [DK, D], F32, kind="ExternalInput")
    y = nc.dram_tensor("y", [B, D], F32, kind="ExternalOutput")

    with tile.TileContext(nc) as tc:
        with (
            tc.tile_pool(name="const", bufs=1) as const,
            tc.tile_pool(name="epool", bufs=11) as epool,
            tc.tile_pool(name="otpool", bufs=2) as otpool,
            tc.tile_pool(name="ypool", bufs=3) as ypool,
            tc.tile_pool(name="rpool", bufs=4) as rpool,
            tc.tile_pool(name="score_ps", bufs=2, space="PSUM") as score_ps,
            tc.tile_pool(name="attnv_ps", bufs=1, space="PSUM") as attnv_ps,
            tc.tile_pool(name="out_ps", bufs=1, space="PSUM") as out_ps,
        ):
            # ---- persistent SBUF ----
            x_sb = const.tile([128, 4 * B], F32)  # 4 c-chunks side by side
            wq_sb = const.tile([128, 512], F32)  # 4 c-chunks of [128,128]
            wk_sb = const.tile([128, 512], F32)
            wv_sb = const.tile([128, 4 * DK], F32)  # 4 c-chunks of [128,64]
            bq_sb = const.tile([128, 1], F32)
            bk_sb = const.tile([128, 1], F32)
            bv_sb = const.tile([1, DK], F32)
            wot_sb = const.tile([DK, D], F32)
            ones_sb = const.tile([128, 1], F32)
            onesr_sb = const.tile([1, 128], F32)
            qt_sb = const.tile([128, B], F32)  # QT dup'd across partition halves
            kt_sb = const.tile([128, B], F32)
            vp_sb = const.tile([128, JB * (DK + 1)], F32)  # [V | 1] per j-block

            # ---- input DMAs ----
            for c in range(4):
                for n in range(N_MC):
                    nc.sync.dma_start(
                        out=x_sb[:, c * B + n * MC : c * B + (n + 1) * MC],
                        in_=xt[c * 128 : (c + 1) * 128, n * MC : (n + 1) * MC],
                    )
            for c in range(4):
                nc.sync.dma_start(out=wq_sb[:, c * 128 : (c + 1) * 128], in_=wqt[c * 128 : (c + 1) * 128, :])
                nc.sync.dma_start(out=wk_sb[:, c * 128 : (c + 1) * 128], in_=wkt[c * 128 : (c + 1) * 128, :])
                nc.sync.dma_start(out=wv_sb[:, c * DK : (c + 1) * DK], in_=wvt[c * 128 : (c + 1) * 128, :])
            nc.sync.dma_start(out=bq_sb[:], in_=bqd[:])
            nc.sync.dma_start(out=bk_sb[:], in_=bkd[:])
            nc.sync.dma_start(out=bv_sb[:], in_=bvr[:])
            nc.sync.dma_start(out=wot_sb[:], in_=wot[:])
            nc.vector.memset(ones_sb[:], 1.0)
            nc.vector.memset(onesr_sb[:], 1.0)
            nc.vector.memset(vp_sb[:], 1.0)  # ones columns; V data overwrites the rest

            # ---- Q/K projections: QT/KT [128(d dup), 4096] = (Wq dup).T-contract @ xT ----
            for n in range(N_MC):
                for w_sb, b_sb, dst in ((wq_sb, bq_sb, qt_sb), (wk_sb, bk_sb, kt_sb)):
                    pp = score_ps.tile([128, MC], F32, tag="score")
                    for c in range(4):
                        nc.tensor.matmul(
                            pp[:],
                            w_sb[:, c * 128 : (c + 1) * 128],
                            x_sb[:, c * B + n * MC : c * B + (n + 1) * MC],
                            start=(c == 0),
                            stop=(c == 3),
                        )
                    nc.vector.tensor_scalar(
                        out=dst[:, n * MC : (n + 1) * MC], in0=pp[:], scalar1=b_sb[:],
                        scalar2=None, op0=mybir.AluOpType.add,
                    )

            # ---- V projection, natural layout, bias via ones-row matmul ----
            for t in range(JB):
                vps = score_ps.tile([128, DK], F32, tag="score")
                for c in range(4):
                    nc.tensor.matmul(
                        vps[:],
                        x_sb[:, c * B + t * 128 : c * B + (t + 1) * 128],
                        wv_sb[:, c * DK : (c + 1) * DK],
                        start=(c == 0),
                        stop=False,
                    )
                nc.tensor.matmul(vps[:], onesr_sb[:], bv_sb[:], start=False, stop=True)
                nc.vector.tensor_copy(vp_sb[:, t * (DK + 1) : t * (DK + 1) + DK], vps[:])

            # ---- main loop over query chunks ----
            for mc in range(N_MC):
                m0 = mc * MC
                av = attnv_ps.tile([DK + 1, MC], F32, tag="attnv")
                for g0, gn in JGROUPS:
                    sp = score_ps.tile([128, gn * MC], F32, tag="score")
                    et = epool.tile([128, gn * MC], F32, tag="E")
                    for k in range(gn):
                        jb = g0 + k
                        h0 = 64 * (jb % 2)
                        nc.tensor.matmul(
                            sp[:, k * MC : (k + 1) * MC],
                            kt_sb[h0 : h0 + 64, jb * 128 : (jb + 1) * 128],
                            qt_sb[h0 : h0 + 64, m0 : m0 + MC],
                            start=True,
                            stop=True,
                        )
                    nc.scalar.activation(et[:], sp[:], mybir.ActivationFunctionType.Exp, scale=0.125)
                    for k in range(gn):
                        jb = g0 + k
                        nc.tensor.matmul(
                            av[:],
                            vp_sb[:, jb * (DK + 1) : (jb + 1) * (DK + 1)],
                            et[:, k * MC : (k + 1) * MC],
                            start=(jb == 0),
                            stop=(jb == JB - 1),
                        )
                ot = otpool.tile([DK + 1, MC], F32, tag="ot")
                nc.vector.tensor_copy(ot[:], av[:])
                for q in range(4):
                    rt = out_ps.tile([128, MC], F32, tag="out")
                    nc.tensor.matmul(
                        rt[:, 0:1],
                        ot[DK : DK + 1, q * 128 : (q + 1) * 128],
                        ones_sb[DK : DK + 1, 0:1],
                        start=True,
                        stop=True,
                    )
                    rv = rpool.tile([128, 1], F32, tag="rinv")
                    nc.vector.reciprocal(rv[:], rt[:, 0:1])
                    yp = out_ps.tile([128, MC], F32, tag="out")
                    nc.tensor.matmul(yp[:], ot[0:DK, q * 128 : (q + 1) * 128], wot_sb[:], start=True, stop=True)
                    ysb = ypool.tile([128, MC], F32, tag="y")
                    nc.vector.tensor_scalar(
                        out=ysb[:], in0=yp[:], scalar1=rv[:], scalar2=None, op0=mybir.AluOpType.mult
                    )
                    nc.sync.dma_start(out=y[m0 + q * 128 : m0 + (q + 1) * 128, :], in_=ysb[:])
    nc.finalize()
    return nc


def _get_nc():
    global _NC_CACHE
    if _NC_CACHE is None:
        _NC_CACHE = build_nc()
    return _NC_CACHE


def make_in_maps(x, Wq, bq, Wk, bk, Wv, bv, Wo, bo):
    xT = np.ascontiguousarray(np.asarray(x, dtype=np.float32).T)
    maps = []
    for h in range(H):
        s = slice(h * DK, (h + 1) * DK)
        wqT = np.asarray(Wq, np.float32)[s, :].T  # [512, 64]
        wkT = np.asarray(Wk, np.float32)[s, :].T
        maps.append(
            {
                "xt": xT,
                "wqt": np.ascontiguousarray(np.concatenate([wqT, wqT], axis=1)),
                "wkt": np.ascontiguousarray(np.concatenate([wkT, wkT], axis=1)),
                "wvt": np.ascontiguousarray(np.asarray(Wv, np.float32)[s, :].T),
                "bqd": np.ascontiguousarray(np.tile(np.asarray(bq, np.float32)[s], 2).reshape(128, 1)),
                "bkd": np.ascontiguousarray(np.tile(np.asarray(bk, np.float32)[s], 2).reshape(128, 1)),
                "bvr": np.ascontiguousarray(np.asarray(bv, np.float32)[s].reshape(1, DK)),
                "wot": np.ascontiguousarray(np.asarray(Wo, np.float32)[:, s].T),
            }
        )
    return maps


def run(inputs, trace=False, **kw):
    from concourse.bass_utils import run_bass_kernel_spmd

    nc = _get_nc()
    in_maps = make_in_maps(**inputs)
    res = run_bass_kernel_spmd(nc, in_maps, list(range(H)), trace=trace, **kw)
    bo = np.asarray(inputs["bo"], np.float32)
    out = np.zeros((B, D), np.float32)
    for c in range(H):
        out += res.results[c]["y"]
    out += bo[None, :]
    return out, res


def kernel(**inputs):
    out, _ = run(inputs, trace=False)
    return out


# revision 6
# speedup vs baseline: 2.7558x; 2.7558x over previous
"""Multi-head cross-batch attention (B=4096, d_model=512, H=8 heads) on 8 TRN2 cores.

Sharding: one head per NeuronCore (tensor-parallel over H). Each core computes
its head's Q/K/V projections from a replicated (pre-transposed) x, the full
[4096, 4096] score block for that head, softmax (transposed layout, denominator
via a ones-column in V), attn @ V, and its partial out-projection
Y_h = attn_h @ Wo[:, h*64:(h+1)*64].T. Host sums the 8 partials and adds bo.

Layout notes (per core):
  - xT [512, 4096] (c on partitions) is fed from host so every matmul can
    contract over the partition dim without any on-device transpose of x.
  - QT/KT are stored duplicated across partition halves ([128, 4096]) so score
    matmuls can be row-packed two-at-a-time into the 128x128 PE array (the
    contraction dim is only 64).
  - Scores are computed transposed (ST[j, m]) so softmax's sum over keys j can
    ride the attn@V matmul: V is augmented with a ones column, making the
    accumulated output row 64 equal to sum_j exp(s). No max-subtraction is
    needed: scores are O(1) here (verified), so exp cannot overflow.
  - Normalization commutes with the out-projection, so Y_un rows are scaled by
    1/r with a per-partition tensor_scalar after the final matmul. r (living in
    a free-dim row) is transposed to partitions with a K=1 matmul.
  - Matmul inputs are bf16 (1 PE pass vs 2 for fp32, fast weight load);
    accumulation is fp32 in PSUM, exp inputs and the softmax denominator stay
    fp32. Emulated error of this variant vs the f64 reference: l2 rel 1.9e-3.
"""

import sys

if "/opt/trn_rl_repo" not in sys.path:
    sys.path.insert(0, "/opt/trn_rl_repo")

import ml_dtypes
import numpy as np

import concourse.bass as bass
import concourse.tile as tile
from concourse import bacc, mybir

B = 4096
D = 512
H = 8
DK = 64
MC = 512  # query-chunk (m) width
N_MC = B // MC  # 8
JB = B // 128  # 32 j-blocks of 128 keys
F32 = mybir.dt.float32
BF16 = mybir.dt.bfloat16
MM_DT = BF16
NP_MM_DT = ml_dtypes.bfloat16 if MM_DT == BF16 else np.float32

# j-blocks per score/exp group: 3 blocks = 1536 floats = 3 PSUM banks.
# PSUM budget: 2x3 (score staging) + 1 (attnV accum) + 1 (outproj/rT) = 8 banks.
JGROUPS = [(0, 3), (3, 3), (6, 3), (9, 3), (12, 3), (15, 3), (18, 3), (21, 3), (24, 3), (27, 3), (30, 2)]

_NC_CACHE = None


def build_nc():
    nc = bacc.Bacc()

    xt = nc.dram_tensor("xt", [D, B], MM_DT, kind="ExternalInput")
    wqt = nc.dram_tensor("wqt", [D, 128], MM_DT, kind="ExternalInput")  # [c, d dup'd]
    wkt = nc.dram_tensor("wkt", [D, 128], MM_DT, kind="ExternalInput")
    wvt = nc.dram_tensor("wvt", [D, DK], MM_DT, kind="ExternalInput")  # [c, d]
    bqd = nc.dram_tensor("bqd", [128, 1], F32, kind="ExternalInput")  # bias dup'd
    bkd = nc.dram_tensor("bkd", [128, 1], F32, kind="ExternalInput")
    bvr = nc.dram_tensor("bvr", [1, DK], MM_DT, kind="ExternalInput")  # bias as row
    wot = nc.dram_tensor("wot", [DK, D], MM_DT, kind="ExternalInput")
    y = nc.dram_tensor("y", [B, D], F32, kind="ExternalOutput")

    with tile.TileContext(nc) as tc:
        with (
            tc.tile_pool(name="const", bufs=1) as const,
            tc.tile_pool(name="epool", bufs=11) as epool,
            tc.tile_pool(name="otpool", bufs=2) as otpool,
            tc.tile_pool(name="ypool", bufs=3) as ypool,
            tc.tile_pool(name="rpool", bufs=4) as rpool,
            tc.tile_pool(name="score_ps", bufs=2, space="PSUM") as score_ps,
            tc.tile_pool(name="attnv_ps", bufs=1, space="PSUM") as attnv_ps,
            tc.tile_pool(name="out_ps", bufs=1, space="PSUM") as out_ps,
        ):
            # ---- persistent SBUF ----
            x_sb = const.tile([128, 4 * B], MM_DT)  # 4 c-chunks side by side
            wq_sb = const.tile([128, 512], MM_DT)  # 4 c-chunks of [128,128]
            wk_sb = const.tile([128, 512], MM_DT)
            wv_sb = const.tile([128, 4 * DK], MM_DT)  # 4 c-chunks of [128,64]
            bq_sb = const.tile([128, 1], F32)
            bk_sb = const.tile([128, 1], F32)
            bv_sb = const.tile([1, DK], MM_DT)
            wot_sb = const.tile([DK, D], MM_DT)
            ones_sb = const.tile([128, 1], F32)
            onesr_sb = const.tile([1, 128], MM_DT)
            qt_sb = const.tile([128, B], MM_DT)  # QT dup'd across partition halves
            kt_sb = const.tile([128, B], MM_DT)
            vp_sb = const.tile([128, JB * (DK + 1)], MM_DT)  # [V | 1] per j-block

            # ---- input DMAs ----
            for c in range(4):
                for n in range(N_MC):
                    nc.sync.dma_start(
                        out=x_sb[:, c * B + n * MC : c * B + (n + 1) * MC],
                        in_=xt[c * 128 : (c + 1) * 128, n * MC : (n + 1) * MC],
                    )
            for c in range(4):
                nc.sync.dma_start(out=wq_sb[:, c * 128 : (c + 1) * 128], in_=wqt[c * 128 : (c + 1) * 128, :])
                nc.sync.dma_start(out=wk_sb[:, c * 128 : (c + 1) * 128], in_=wkt[c * 128 : (c + 1) * 128, :])
                nc.sync.dma_start(out=wv_sb[:, c * DK : (c + 1) * DK], in_=wvt[c * 128 : (c + 1) * 128, :])
            nc.sync.dma_start(out=bq_sb[:], in_=bqd[:])
            nc.sync.dma_start(out=bk_sb[:], in_=bkd[:])
            nc.sync.dma_start(out=bv_sb[:], in_=bvr[:])
            nc.sync.dma_start(out=wot_sb[:], in_=wot[:])
            nc.vector.memset(ones_sb[:], 1.0)
            nc.vector.memset(onesr_sb[:], 1.0)
            nc.vector.memset(vp_sb[:], 1.0)  # ones columns; V data overwrites the rest

            # ---- Q/K projections: QT/KT [128(d dup), 4096] ----
            for n in range(N_MC):
                for w_sb, b_sb, dst in ((wq_sb, bq_sb, qt_sb), (wk_sb, bk_sb, kt_sb)):
                    pp = score_ps.tile([128, MC], F32, tag="score")
                    for c in range(4):
                        nc.tensor.matmul(
                            pp[:],
                            w_sb[:, c * 128 : (c + 1) * 128],
                            x_sb[:, c * B + n * MC : c * B + (n + 1) * MC],
                            start=(c == 0),
                            stop=(c == 3),
                        )
                    nc.vector.tensor_scalar(
                        out=dst[:, n * MC : (n + 1) * MC], in0=pp[:], scalar1=b_sb[:],
                        scalar2=None, op0=mybir.AluOpType.add,
                    )

            # ---- V projection, natural layout, bias via ones-row matmul ----
            for t in range(JB):
                vps = score_ps.tile([128, DK], F32, tag="score")
                for c in range(4):
                    nc.tensor.matmul(
                        vps[:],
                        x_sb[:, c * B + t * 128 : c * B + (t + 1) * 128],
                        wv_sb[:, c * DK : (c + 1) * DK],
                        start=(c == 0),
                        stop=False,
                    )
                nc.tensor.matmul(vps[:], onesr_sb[:], bv_sb[:], start=False, stop=True)
                nc.vector.tensor_copy(vp_sb[:, t * (DK + 1) : t * (DK + 1) + DK], vps[:])

            # ---- main loop over query chunks ----
            for mc in range(N_MC):
                m0 = mc * MC
                av = attnv_ps.tile([DK + 1, MC], F32, tag="attnv")
                for g0, gn in JGROUPS:
                    sp = score_ps.tile([128, gn * MC], F32, tag="score")
                    et = epool.tile([128, gn * MC], MM_DT, tag="E")
                    for k in range(gn):
                        jb = g0 + k
                        h0 = 64 * (jb % 2)
                        nc.tensor.matmul(
                            sp[:, k * MC : (k + 1) * MC],
                            kt_sb[h0 : h0 + 64, jb * 128 : (jb + 1) * 128],
                            qt_sb[h0 : h0 + 64, m0 : m0 + MC],
                            start=True,
                            stop=True,
                        )
                    nc.scalar.activation(et[:], sp[:], mybir.ActivationFunctionType.Exp, scale=0.125)
                    for k in range(gn):
                        jb = g0 + k
                        nc.tensor.matmul(
                            av[:],
                            vp_sb[:, jb * (DK + 1) : (jb + 1) * (DK + 1)],
                            et[:, k * MC : (k + 1) * MC],
                            start=(jb == 0),
                            stop=(jb == JB - 1),
                        )
                # r row (f32, feeds the K=1 transpose matmul) + bf16 numerator
                ot_f = otpool.tile([DK + 1, MC], F32, tag="otf")
                nc.vector.tensor_copy(ot_f[DK : DK + 1, :], av[DK : DK + 1, :])
                ot_b = otpool.tile([DK, MC], MM_DT, tag="otb")
                nc.vector.tensor_copy(ot_b[:], av[0:DK, :])
                for q in range(4):
                    rt = out_ps.tile([128, MC], F32, tag="out")
                    nc.tensor.matmul(
                        rt[:, 0:1],
                        ot_f[DK : DK + 1, q * 128 : (q + 1) * 128],
                        ones_sb[DK : DK + 1, 0:1],
                        start=True,
                        stop=True,
                    )
                    rv = rpool.tile([128, 1], F32, tag="rinv")
                    nc.vector.reciprocal(rv[:], rt[:, 0:1])
                    yp = out_ps.tile([128, MC], F32, tag="out")
                    nc.tensor.matmul(yp[:], ot_b[:, q * 128 : (q + 1) * 128], wot_sb[:], start=True, stop=True)
                    ysb = ypool.tile([128, MC], F32, tag="y")
                    nc.vector.tensor_scalar(
                        out=ysb[:], in0=yp[:], scalar1=rv[:], scalar2=None, op0=mybir.AluOpType.mult
                    )
                    nc.sync.dma_start(out=y[m0 + q * 128 : m0 + (q + 1) * 128, :], in_=ysb[:])
    nc.finalize()
    return nc


def _get_nc():
    global _NC_CACHE
    if _NC_CACHE is None:
        _NC_CACHE = build_nc()
    return _NC_CACHE


def make_in_maps(x, Wq, bq, Wk, bk, Wv, bv, Wo, bo):
    xT = np.ascontiguousarray(np.asarray(x, dtype=np.float32).T).astype(NP_MM_DT)
    maps = []
    for h in range(H):
        s = slice(h * DK, (h + 1) * DK)
        wqT = np.asarray(Wq, np.float32)[s, :].T  # [512, 64]
        wkT = np.asarray(Wk, np.float32)[s, :].T
        maps.append(
            {
                "xt": xT,
                "wqt": np.ascontiguousarray(np.concatenate([wqT, wqT], axis=1)).astype(NP_MM_DT),
                "wkt": np.ascontiguousarray(np.concatenate([wkT, wkT], axis=1)).astype(NP_MM_DT),
                "wvt": np.ascontiguousarray(np.asarray(Wv, np.float32)[s, :].T).astype(NP_MM_DT),
                "bqd": np.ascontiguousarray(np.tile(np.asarray(bq, np.float32)[s], 2).reshape(128, 1)),
                "bkd": np.ascontiguousarray(np.tile(np.asarray(bk, np.float32)[s], 2).reshape(128, 1)),
                "bvr": np.ascontiguousarray(np.asarray(bv, np.float32)[s].reshape(1, DK)).astype(NP_MM_DT),
                "wot": np.ascontiguousarray(np.asarray(Wo, np.float32)[:, s].T).astype(NP_MM_DT),
            }
        )
    return maps


def run(inputs, trace=False, **kw):
    from concourse.bass_utils import run_bass_kernel_spmd

    nc = _get_nc()
    in_maps = make_in_maps(**inputs)
    res = run_bass_kernel_spmd(nc, in_maps, list(range(H)), trace=trace, **kw)
    bo = np.asarray(inputs["bo"], np.float32)
    out = np.zeros((B, D), np.float32)
    for c in range(H):
        out += res.results[c]["y"]
    out += bo[None, :]
    return out, res


def kernel(**inputs):
    out, _ = run(inputs, trace=False)
    return out


# revision 8
# speedup vs baseline: 2.7802x; 1.0089x over previous
"""Multi-head cross-batch attention (B=4096, d_model=512, H=8 heads) on 8 TRN2 cores.

Sharding: one head per NeuronCore (tensor-parallel over H). Each core computes
its head's Q/K/V projections from a replicated (pre-transposed) x, the full
[4096, 4096] score block for that head, softmax (transposed layout, denominator
via a ones-column in V), attn @ V, and its partial out-projection
Y_h = attn_h @ Wo[:, h*64:(h+1)*64].T. Host sums the 8 partials and adds bo.

Layout notes (per core):
  - xT [512, 4096] (c on partitions) is fed from host so every matmul can
    contract over the partition dim without any on-device transpose of x.
  - QT/KT are stored duplicated across partition halves ([128, 4096]) so score
    matmuls can be row-packed two-at-a-time into the 128x128 PE array (the
    contraction dim is only 64).
  - Scores are computed transposed (ST[j, m]) so softmax's sum over keys j can
    ride the attn@V matmul: V is augmented with a ones column, making the
    accumulated output row 64 equal to sum_j exp(s). No max-subtraction is
    needed: scores are O(1) here (verified), so exp cannot overflow.
  - Normalization commutes with the out-projection, so Y_un rows are scaled by
    1/r with a per-partition tensor_scalar after the final matmul. r (living in
    a free-dim row) is transposed to partitions with a K=1 matmul.
  - Matmul inputs are bf16 (1 PE pass vs 2 for fp32, fast weight load);
    accumulation is fp32 in PSUM, exp inputs and the softmax denominator stay
    fp32. Emulated error of this variant vs the f64 reference: l2 rel 1.9e-3.
"""

import sys

if "/opt/trn_rl_repo" not in sys.path:
    sys.path.insert(0, "/opt/trn_rl_repo")

import ml_dtypes
import numpy as np

import concourse.bass as bass
import concourse.tile as tile
from concourse import bacc, mybir

B = 4096
D = 512
H = 8
DK = 64
MC = 512  # query-chunk (m) width
N_MC = B // MC  # 8
JB = B // 128  # 32 j-blocks of 128 keys
F32 = mybir.dt.float32
BF16 = mybir.dt.bfloat16
MM_DT = BF16
NP_MM_DT = ml_dtypes.bfloat16 if MM_DT == BF16 else np.float32

# j-blocks per score/exp group: 3 blocks = 1536 floats = 3 PSUM banks.
# PSUM budget: 2x3 (score staging) + 1 (attnV accum) + 1 (outproj/rT) = 8 banks.
JGROUPS = [(0, 3), (3, 3), (6, 3), (9, 3), (12, 3), (15, 3), (18, 3), (21, 3), (24, 3), (27, 3), (30, 2)]

_NC_CACHE = None


def build_nc():
    nc = bacc.Bacc()

    xt = nc.dram_tensor("xt", [D, B], MM_DT, kind="ExternalInput")
    wqt = nc.dram_tensor("wqt", [D, 128], MM_DT, kind="ExternalInput")  # [c, d dup'd]
    wkt = nc.dram_tensor("wkt", [D, 128], MM_DT, kind="ExternalInput")
    wvt = nc.dram_tensor("wvt", [D, DK], MM_DT, kind="ExternalInput")  # [c, d]
    bqd = nc.dram_tensor("bqd", [128, 1], F32, kind="ExternalInput")  # bias dup'd
    bkd = nc.dram_tensor("bkd", [128, 1], F32, kind="ExternalInput")
    bvr = nc.dram_tensor("bvr", [1, DK], MM_DT, kind="ExternalInput")  # bias as row
    wot = nc.dram_tensor("wot", [DK, D], MM_DT, kind="ExternalInput")
    y = nc.dram_tensor("y", [B, D], F32, kind="ExternalOutput")

    with tile.TileContext(nc) as tc:
        with (
            tc.tile_pool(name="const", bufs=1) as const,
            tc.tile_pool(name="epool", bufs=16) as epool,
            tc.tile_pool(name="otpool", bufs=2) as otpool,
            tc.tile_pool(name="ypool", bufs=3) as ypool,
            tc.tile_pool(name="rpool", bufs=4) as rpool,
            tc.tile_pool(name="score_ps", bufs=2, space="PSUM") as score_ps,
            tc.tile_pool(name="attnv_ps", bufs=1, space="PSUM") as attnv_ps,
            tc.tile_pool(name="out_ps", bufs=1, space="PSUM") as out_ps,
        ):
            # ---- persistent SBUF ----
            x_sb = const.tile([128, 4 * B], MM_DT)  # 4 c-chunks side by side
            wq_sb = const.tile([128, 512], MM_DT)  # 4 c-chunks of [128,128]
            wk_sb = const.tile([128, 512], MM_DT)
            wv_sb = const.tile([128, 4 * DK], MM_DT)  # 4 c-chunks of [128,64]
            bq_sb = const.tile([128, 1], F32)
            bk_sb = const.tile([128, 1], F32)
            bv_sb = const.tile([1, DK], MM_DT)
            wot_sb = const.tile([DK, D], MM_DT)
            ones_sb = const.tile([128, 1], F32)
            onesr_sb = const.tile([1, 128], MM_DT)
            qt_sb = const.tile([128, B], MM_DT)  # QT dup'd across partition halves
            kt_sb = const.tile([128, B], MM_DT)
            vp_sb = const.tile([128, JB * (DK + 1)], MM_DT)  # [V | 1] per j-block

            # ---- input DMAs ----
            # n-major order: all 4 c-chunks of a column block arrive together, so
            # the projections (and then scores/exp) start after ~1/8 of the x DMA.
            for n in range(N_MC):
                for c in range(4):
                    nc.sync.dma_start(
                        out=x_sb[:, c * B + n * MC : c * B + (n + 1) * MC],
                        in_=xt[c * 128 : (c + 1) * 128, n * MC : (n + 1) * MC],
                    )
            for c in range(4):
                nc.sync.dma_start(out=wq_sb[:, c * 128 : (c + 1) * 128], in_=wqt[c * 128 : (c + 1) * 128, :])
                nc.sync.dma_start(out=wk_sb[:, c * 128 : (c + 1) * 128], in_=wkt[c * 128 : (c + 1) * 128, :])
                nc.sync.dma_start(out=wv_sb[:, c * DK : (c + 1) * DK], in_=wvt[c * 128 : (c + 1) * 128, :])
            nc.sync.dma_start(out=bq_sb[:], in_=bqd[:])
            nc.sync.dma_start(out=bk_sb[:], in_=bkd[:])
            nc.sync.dma_start(out=bv_sb[:], in_=bvr[:])
            nc.sync.dma_start(out=wot_sb[:], in_=wot[:])
            nc.vector.memset(ones_sb[:], 1.0)
            nc.vector.memset(onesr_sb[:], 1.0)
            nc.vector.memset(vp_sb[:], 1.0)  # ones columns; V data overwrites the rest

            # ---- Q/K projections: QT/KT [128(d dup), 4096] ----
            for n in range(N_MC):
                for w_sb, b_sb, dst in ((wq_sb, bq_sb, qt_sb), (wk_sb, bk_sb, kt_sb)):
                    pp = score_ps.tile([128, MC], F32, tag="score")
                    for c in range(4):
                        nc.tensor.matmul(
                            pp[:],
                            w_sb[:, c * 128 : (c + 1) * 128],
                            x_sb[:, c * B + n * MC : c * B + (n + 1) * MC],
                            start=(c == 0),
                            stop=(c == 3),
                        )
                    nc.vector.tensor_scalar(
                        out=dst[:, n * MC : (n + 1) * MC], in0=pp[:], scalar1=b_sb[:],
                        scalar2=None, op0=mybir.AluOpType.add,
                    )

            # ---- V projection, natural layout, bias via ones-row matmul ----
            for t in range(JB):
                vps = score_ps.tile([128, DK], F32, tag="score")
                for c in range(4):
                    nc.tensor.matmul(
                        vps[:],
                        x_sb[:, c * B + t * 128 : c * B + (t + 1) * 128],
                        wv_sb[:, c * DK : (c + 1) * DK],
                        start=(c == 0),
                        stop=False,
                    )
                nc.tensor.matmul(vps[:], onesr_sb[:], bv_sb[:], start=False, stop=True)
                nc.vector.tensor_copy(vp_sb[:, t * (DK + 1) : t * (DK + 1) + DK], vps[:])

            # ---- main loop over query chunks ----
            for mc in range(N_MC):
                m0 = mc * MC
                av = attnv_ps.tile([DK + 1, MC], F32, tag="attnv")
                for g0, gn in JGROUPS:
                    sp = score_ps.tile([128, gn * MC], F32, tag="score")
                    et = epool.tile([128, gn * MC], MM_DT, tag="E")
                    for k in range(gn):
                        jb = g0 + k
                        h0 = 64 * (jb % 2)
                        nc.tensor.matmul(
                            sp[:, k * MC : (k + 1) * MC],
                            kt_sb[h0 : h0 + 64, jb * 128 : (jb + 1) * 128],
                            qt_sb[h0 : h0 + 64, m0 : m0 + MC],
                            start=True,
                            stop=True,
                        )
                    nc.scalar.activation(et[:], sp[:], mybir.ActivationFunctionType.Exp, scale=0.125)
                    for k in range(gn):
                        jb = g0 + k
                        nc.tensor.matmul(
                            av[:],
                            vp_sb[:, jb * (DK + 1) : (jb + 1) * (DK + 1)],
                            et[:, k * MC : (k + 1) * MC],
                            start=(jb == 0),
                            stop=(jb == JB - 1),
                        )
                # r row (f32, feeds the K=1 transpose matmul) + bf16 numerator
                ot_f = otpool.tile([DK + 1, MC], F32, tag="otf")
                nc.vector.tensor_copy(ot_f[DK : DK + 1, :], av[DK : DK + 1, :])
                ot_b = otpool.tile([DK, MC], MM_DT, tag="otb")
                nc.vector.tensor_copy(ot_b[:], av[0:DK, :])
                for q in range(4):
                    rt = out_ps.tile([128, MC], F32, tag="out")
                    nc.tensor.matmul(
                        rt[:, 0:1],
                        ot_f[DK : DK + 1, q * 128 : (q + 1) * 128],
                        ones_sb[DK : DK + 1, 0:1],
                        start=True,
                        stop=True,
                    )
                    rv = rpool.tile([128, 1], F32, tag="rinv")
                    nc.vector.reciprocal(rv[:], rt[:, 0:1])
                    yp = out_ps.tile([128, MC], F32, tag="out")
                    nc.tensor.matmul(yp[:], ot_b[:, q * 128 : (q + 1) * 128], wot_sb[:], start=True, stop=True)
                    ysb = ypool.tile([128, MC], F32, tag="y")
                    nc.vector.tensor_scalar(
                        out=ysb[:], in0=yp[:], scalar1=rv[:], scalar2=None, op0=mybir.AluOpType.mult
                    )
                    nc.sync.dma_start(out=y[m0 + q * 128 : m0 + (q + 1) * 128, :], in_=ysb[:])
    nc.finalize()
    return nc


def _get_nc():
    global _NC_CACHE
    if _NC_CACHE is None:
        _NC_CACHE = build_nc()
    return _NC_CACHE


def make_in_maps(x, Wq, bq, Wk, bk, Wv, bv, Wo, bo):
    xT = np.ascontiguousarray(np.asarray(x, dtype=np.float32).T).astype(NP_MM_DT)
    maps = []
    for h in range(H):
        s = slice(h * DK, (h + 1) * DK)
        wqT = np.asarray(Wq, np.float32)[s, :].T  # [512, 64]
        wkT = np.asarray(Wk, np.float32)[s, :].T
        maps.append(
            {
                "xt": xT,
                "wqt": np.ascontiguousarray(np.concatenate([wqT, wqT], axis=1)).astype(NP_MM_DT),
                "wkt": np.ascontiguousarray(np.concatenate([wkT, wkT], axis=1)).astype(NP_MM_DT),
                "wvt": np.ascontiguousarray(np.asarray(Wv, np.float32)[s, :].T).astype(NP_MM_DT),
                "bqd": np.ascontiguousarray(np.tile(np.asarray(bq, np.float32)[s], 2).reshape(128, 1)),
                "bkd": np.ascontiguousarray(np.tile(np.asarray(bk, np.float32)[s], 2).reshape(128, 1)),
                "bvr": np.ascontiguousarray(np.asarray(bv, np.float32)[s].reshape(1, DK)).astype(NP_MM_DT),
                "wot": np.ascontiguousarray(np.asarray(Wo, np.float32)[:, s].T).astype(NP_MM_DT),
            }
        )
    return maps


def run(inputs, trace=False, **kw):
    from concourse.bass_utils import run_bass_kernel_spmd

    nc = _get_nc()
    in_maps = make_in_maps(**inputs)
    res = run_bass_kernel_spmd(nc, in_maps, list(range(H)), trace=trace, **kw)
    bo = np.asarray(inputs["bo"], np.float32)
    out = np.zeros((B, D), np.float32)
    for c in range(H):
        out += res.results[c]["y"]
    out += bo[None, :]
    return out, res


def kernel(**inputs):
    out, _ = run(inputs, trace=False)
    return out


# revision 9
# speedup vs baseline: 3.0126x; 1.0836x over previous
"""Multi-head cross-batch attention (B=4096, d_model=512, H=8 heads) on 8 TRN2 cores.

Sharding: one head per NeuronCore (tensor-parallel over H). Each core computes
its head's Q/K/V projections from a replicated (pre-transposed) x, the full
[4096, 4096] score block for that head, softmax (transposed layout, denominator
via a ones-column in V), attn @ V, and its partial out-projection
Y_h = attn_h @ Wo[:, h*64:(h+1)*64].T. Host sums the 8 partials and adds bo.

Layout notes (per core):
  - xT [512, 4096] (c on partitions) is fed from host so every matmul can
    contract over the partition dim without any on-device transpose of x.
  - QT/KT are stored duplicated across partition halves ([128, 4096]) so score
    matmuls can be row-packed two-at-a-time into the 128x128 PE array (the
    contraction dim is only 64).
  - Scores are computed transposed (ST[j, m]) so softmax's sum over keys j can
    ride the attn@V matmul: V is augmented with a ones column, making the
    accumulated output row 64 equal to sum_j exp(s). No max-subtraction is
    needed: scores are O(1) here (verified), so exp cannot overflow.
  - Normalization commutes with the out-projection, so Y_un rows are scaled by
    1/r with a per-partition tensor_scalar after the final matmul. r (living in
    a free-dim row) is transposed to partitions with a K=1 matmul.
  - Matmul inputs are bf16 (1 PE pass vs 2 for fp32, fast weight load);
    accumulation is fp32 in PSUM, exp inputs and the softmax denominator stay
    fp32. Emulated error of this variant vs the f64 reference: l2 rel 1.9e-3.
"""

import sys

if "/opt/trn_rl_repo" not in sys.path:
    sys.path.insert(0, "/opt/trn_rl_repo")

import ml_dtypes
import numpy as np

import concourse.bass as bass
import concourse.tile as tile
from concourse import bacc, mybir

B = 4096
D = 512
H = 8
DK = 64
MC = 512  # query-chunk (m) width
N_MC = B // MC  # 8
JB = B // 128  # 32 j-blocks of 128 keys
F32 = mybir.dt.float32
BF16 = mybir.dt.bfloat16
MM_DT = BF16
NP_MM_DT = ml_dtypes.bfloat16 if MM_DT == BF16 else np.float32

# j-blocks per score/exp group: 3 blocks = 1536 floats = 3 PSUM banks.
# PSUM budget: 2x3 (score staging) + 1 (attnV accum) + 1 (outproj/rT) = 8 banks.
JGROUPS = [(0, 3), (3, 3), (6, 3), (9, 3), (12, 3), (15, 3), (18, 3), (21, 3), (24, 3), (27, 3), (30, 2)]

_NC_CACHE = None


def build_nc():
    nc = bacc.Bacc()

    xt = nc.dram_tensor("xt", [D, B], MM_DT, kind="ExternalInput")
    wqt = nc.dram_tensor("wqt", [D, 128], MM_DT, kind="ExternalInput")  # [c, d dup'd]
    wkt = nc.dram_tensor("wkt", [D, 128], MM_DT, kind="ExternalInput")
    wvt = nc.dram_tensor("wvt", [D, DK], MM_DT, kind="ExternalInput")  # [c, d]
    bqd = nc.dram_tensor("bqd", [128, 1], F32, kind="ExternalInput")  # bias dup'd
    bkd = nc.dram_tensor("bkd", [128, 1], F32, kind="ExternalInput")
    bvr = nc.dram_tensor("bvr", [1, DK], MM_DT, kind="ExternalInput")  # bias as row
    wot = nc.dram_tensor("wot", [DK, D], MM_DT, kind="ExternalInput")
    y = nc.dram_tensor("y", [B, D], F32, kind="ExternalOutput")

    with tile.TileContext(nc) as tc:
        with (
            tc.tile_pool(name="const", bufs=1) as const,
            tc.tile_pool(name="epool", bufs=16) as epool,
            tc.tile_pool(name="otpool", bufs=2) as otpool,
            tc.tile_pool(name="ypool", bufs=3) as ypool,
            tc.tile_pool(name="rpool", bufs=4) as rpool,
            tc.tile_pool(name="score_ps", bufs=2, space="PSUM") as score_ps,
            tc.tile_pool(name="attnv_ps", bufs=1, space="PSUM") as attnv_ps,
            tc.tile_pool(name="out_ps", bufs=1, space="PSUM") as out_ps,
        ):
            # ---- persistent SBUF ----
            x_sb = const.tile([128, 4 * B], MM_DT)  # 4 c-chunks side by side
            wq_sb = const.tile([128, 512], MM_DT)  # 4 c-chunks of [128,128]
            wk_sb = const.tile([128, 512], MM_DT)
            wv_sb = const.tile([128, 4 * DK], MM_DT)  # 4 c-chunks of [128,64]
            bq_sb = const.tile([128, 1], F32)
            bk_sb = const.tile([128, 1], F32)
            bv_sb = const.tile([1, DK], MM_DT)
            wot_sb = const.tile([DK, D], MM_DT)
            ones_sb = const.tile([128, 1], F32)
            onesr_sb = const.tile([1, 128], MM_DT)
            qt_sb = const.tile([128, B], MM_DT)  # QT dup'd across partition halves
            kt_sb = const.tile([128, B], MM_DT)
            vp_sb = const.tile([128, JB * (DK + 1)], MM_DT)  # [V | 1] per j-block

            # ---- input DMAs ----
            # Weights first (tiny, gate every projection matmul), one descriptor
            # per tensor via 3D APs. x streams n-major as 8 consolidated DMAs on
            # GpSimd so projections/scores/exp start after ~1/8 of the x DMA.
            nc.sync.dma_start(
                out=wq_sb[:].rearrange("p (c n) -> p c n", c=4),
                in_=wqt[:].rearrange("(c p) n -> p c n", p=128),
            )
            nc.sync.dma_start(
                out=wk_sb[:].rearrange("p (c n) -> p c n", c=4),
                in_=wkt[:].rearrange("(c p) n -> p c n", p=128),
            )
            nc.sync.dma_start(
                out=wv_sb[:].rearrange("p (c n) -> p c n", c=4),
                in_=wvt[:].rearrange("(c p) n -> p c n", p=128),
            )
            nc.sync.dma_start(out=bq_sb[:], in_=bqd[:])
            nc.sync.dma_start(out=bk_sb[:], in_=bkd[:])
            nc.sync.dma_start(out=bv_sb[:], in_=bvr[:])
            nc.sync.dma_start(out=wot_sb[:], in_=wot[:])
            x_sb3 = x_sb[:].rearrange("p (c n) -> p c n", c=4)
            xt3 = xt[:].rearrange("(c p) n -> p c n", p=128)
            for n in range(N_MC):
                nc.gpsimd.dma_start(
                    out=x_sb3[:, :, n * MC : (n + 1) * MC],
                    in_=xt3[:, :, n * MC : (n + 1) * MC],
                )
            nc.vector.memset(ones_sb[:], 1.0)
            nc.vector.memset(onesr_sb[:], 1.0)
            nc.vector.memset(vp_sb[:], 1.0)  # ones columns; V data overwrites the rest

            # ---- Q/K projections: QT/KT [128(d dup), 4096] ----
            for n in range(N_MC):
                for w_sb, b_sb, dst in ((wq_sb, bq_sb, qt_sb), (wk_sb, bk_sb, kt_sb)):
                    pp = score_ps.tile([128, MC], F32, tag="score")
                    for c in range(4):
                        nc.tensor.matmul(
                            pp[:],
                            w_sb[:, c * 128 : (c + 1) * 128],
                            x_sb[:, c * B + n * MC : c * B + (n + 1) * MC],
                            start=(c == 0),
                            stop=(c == 3),
                        )
                    nc.vector.tensor_scalar(
                        out=dst[:, n * MC : (n + 1) * MC], in0=pp[:], scalar1=b_sb[:],
                        scalar2=None, op0=mybir.AluOpType.add,
                    )

            # ---- V projection, natural layout, bias via ones-row matmul ----
            for t in range(JB):
                vps = score_ps.tile([128, DK], F32, tag="score")
                for c in range(4):
                    nc.tensor.matmul(
                        vps[:],
                        x_sb[:, c * B + t * 128 : c * B + (t + 1) * 128],
                        wv_sb[:, c * DK : (c + 1) * DK],
                        start=(c == 0),
                        stop=False,
                    )
                nc.tensor.matmul(vps[:], onesr_sb[:], bv_sb[:], start=False, stop=True)
                nc.vector.tensor_copy(vp_sb[:, t * (DK + 1) : t * (DK + 1) + DK], vps[:])

            # ---- main loop over query chunks ----
            for mc in range(N_MC):
                m0 = mc * MC
                av = attnv_ps.tile([DK + 1, MC], F32, tag="attnv")
                for g0, gn in JGROUPS:
                    sp = score_ps.tile([128, gn * MC], F32, tag="score")
                    et = epool.tile([128, gn * MC], MM_DT, tag="E")
                    for k in range(gn):
                        jb = g0 + k
                        h0 = 64 * (jb % 2)
                        nc.tensor.matmul(
                            sp[:, k * MC : (k + 1) * MC],
                            kt_sb[h0 : h0 + 64, jb * 128 : (jb + 1) * 128],
                            qt_sb[h0 : h0 + 64, m0 : m0 + MC],
                            start=True,
                            stop=True,
                        )
                    nc.scalar.activation(et[:], sp[:], mybir.ActivationFunctionType.Exp, scale=0.125)
                    for k in range(gn):
                        jb = g0 + k
                        nc.tensor.matmul(
                            av[:],
                            vp_sb[:, jb * (DK + 1) : (jb + 1) * (DK + 1)],
                            et[:, k * MC : (k + 1) * MC],
                            start=(jb == 0),
                            stop=(jb == JB - 1),
                        )
                # r row (f32, feeds the K=1 transpose matmul) + bf16 numerator
                ot_f = otpool.tile([DK + 1, MC], F32, tag="otf")
                nc.vector.tensor_copy(ot_f[DK : DK + 1, :], av[DK : DK + 1, :])
                ot_b = otpool.tile([DK, MC], MM_DT, tag="otb")
                nc.vector.tensor_copy(ot_b[:], av[0:DK, :])
                for q in range(4):
                    rt = out_ps.tile([128, MC], F32, tag="out")
                    nc.tensor.matmul(
                        rt[:, 0:1],
                        ot_f[DK : DK + 1, q * 128 : (q + 1) * 128],
                        ones_sb[DK : DK + 1, 0:1],
                        start=True,
                        stop=True,
                    )
                    rv = rpool.tile([128, 1], F32, tag="rinv")
                    nc.vector.reciprocal(rv[:], rt[:, 0:1])
                    yp = out_ps.tile([128, MC], F32, tag="out")
                    nc.tensor.matmul(yp[:], ot_b[:, q * 128 : (q + 1) * 128], wot_sb[:], start=True, stop=True)
                    ysb = ypool.tile([128, MC], F32, tag="y")
                    nc.vector.tensor_scalar(
                        out=ysb[:], in0=yp[:], scalar1=rv[:], scalar2=None, op0=mybir.AluOpType.mult
                    )
                    nc.sync.dma_start(out=y[m0 + q * 128 : m0 + (q + 1) * 128, :], in_=ysb[:])
    nc.finalize()
    return nc


def _get_nc():
    global _NC_CACHE
    if _NC_CACHE is None:
        _NC_CACHE = build_nc()
    return _NC_CACHE


def make_in_maps(x, Wq, bq, Wk, bk, Wv, bv, Wo, bo):
    xT = np.ascontiguousarray(np.asarray(x, dtype=np.float32).T).astype(NP_MM_DT)
    maps = []
    for h in range(H):
        s = slice(h * DK, (h + 1) * DK)
        wqT = np.asarray(Wq, np.float32)[s, :].T  # [512, 64]
        wkT = np.asarray(Wk, np.float32)[s, :].T
        maps.append(
            {
                "xt": xT,
                "wqt": np.ascontiguousarray(np.concatenate([wqT, wqT], axis=1)).astype(NP_MM_DT),
                "wkt": np.ascontiguousarray(np.concatenate([wkT, wkT], axis=1)).astype(NP_MM_DT),
                "wvt": np.ascontiguousarray(np.asarray(Wv, np.float32)[s, :].T).astype(NP_MM_DT),
                "bqd": np.ascontiguousarray(np.tile(np.asarray(bq, np.float32)[s], 2).reshape(128, 1)),
                "bkd": np.ascontiguousarray(np.tile(np.asarray(bk, np.float32)[s], 2).reshape(128, 1)),
                "bvr": np.ascontiguousarray(np.asarray(bv, np.float32)[s].reshape(1, DK)).astype(NP_MM_DT),
                "wot": np.ascontiguousarray(np.asarray(Wo, np.float32)[:, s].T).astype(NP_MM_DT),
            }
        )
    return maps


def run(inputs, trace=False, **kw):
    from concourse.bass_utils import run_bass_kernel_spmd

    nc = _get_nc()
    in_maps = make_in_maps(**inputs)
    res = run_bass_kernel_spmd(nc, in_maps, list(range(H)), trace=trace, **kw)
    bo = np.asarray(inputs["bo"], np.float32)
    out = np.zeros((B, D), np.float32)
    for c in range(H):
        out += res.results[c]["y"]
    out += bo[None, :]
    return out, res


def kernel(**inputs):
    out, _ = run(inputs, trace=False)
    return out
